# revision 1
# baseline (speedup 1.0000x reference)
"""DeepFM (embedding_lookup) Trainium2 Bass kernel.

Gather strategy: the embedding+first table is stored as 256-byte "quad"
rows (4 consecutive vocab entries x 32 bf16 each: [16 emb | first | 15
pad]). Per feature, one InstDMAGatherAnt gathers all 2048 samples' quads
(int16 quad index = v>>2 fits: 25000 < 32767). The right entry of the 4
candidates is then selected on-chip with host-built masks via
copy_predicated. This replaces 416 serialized indirect DMAs (1 us each)
with 26 gathers.

Data-parallel on batch across 8 cores; table replicated.
"""

import numpy as np
import ml_dtypes

import concourse.bass as bass
import concourse.bacc as bacc
import concourse.mybir as mybir
import concourse.tile as tile
from concourse.bass_utils import run_bass_kernel_spmd
from concourse.masks import make_identity

F32 = mybir.dt.float32
BF16 = mybir.dt.bfloat16
I32 = mybir.dt.int32
I16 = mybir.dt.int16
U8 = mybir.dt.uint8
AF = mybir.ActivationFunctionType
ALU = mybir.AluOpType

B, NCONT, F, V, D = 16384, 13, 26, 100000, 16
H1, H2 = 400, 400
NCORES = 8
BC = B // NCORES          # 2048 rows per core
SUB = 128
NSUB = 4
BLK = SUB * NSUB          # 512
NBLK = BC // BLK          # 4
NSB = NBLK * NSUB         # 16 sub-blocks of 128 rows per core
W17 = D + 1
GW = F * W17              # 442
XW = NCONT + GW           # 455
VQ = V // 4               # 25000 quad rows per feature
EQ = 128                  # quad row: 4 x 32 bf16 = 256B
NG = 2                    # gather chunks per feature (1024 idxs each)
NGATHER = NG * F          # 52 gathers per core
QASSIGN = [g % 4 for g in range(NGATHER)]
# idx partition base per queue (probed on HW): queue q reads partitions
# QPART[q] .. QPART[q]+15 of the idxs AP
QPART = [16, 48, 80, 112]


def _chunks(total, step=128):
    return [(s, min(step, total - s)) for s in range(0, total, step)]


def build_kernel():
    KCH = _chunks(XW)
    MCH1 = _chunks(H1)
    MCH2 = _chunks(H2)
    n_wo_ch = len(MCH2)

    nc = bacc.Bacc("TRN2", target_bir_lowering=False, debug=False,
                   dynamic_dma_scratch_size=32768, num_swdge_queues=4)

    t_table = nc.dram_tensor("table", [F * VQ, EQ], BF16, kind="ExternalInput")
    t_idx = nc.dram_tensor("idx", [128, F * (BC // 16)], I16, kind="ExternalInput")
    t_mask = nc.dram_tensor("mask", [128, 4 * NSB * F], U8, kind="ExternalInput")
    t_cont = nc.dram_tensor("cont", [BC, NCONT], BF16, kind="ExternalInput")
    t_w1 = nc.dram_tensor("w1p", [XW, H1], BF16, kind="ExternalInput")
    t_w2 = nc.dram_tensor("w2", [H1, H2], BF16, kind="ExternalInput")
    t_b1 = nc.dram_tensor("b1", [H1, 1], F32, kind="ExternalInput")
    t_b2 = nc.dram_tensor("b2", [H2, 1], F32, kind="ExternalInput")
    t_wo = nc.dram_tensor("wo", [128, n_wo_ch], BF16, kind="ExternalInput")
    t_wc = nc.dram_tensor("wc", [128, NCONT], F32, kind="ExternalInput")
    t_fs = nc.dram_tensor("fs", [128, 1], F32, kind="ExternalInput")
    t_ob = nc.dram_tensor("ob", [1, 1], F32, kind="ExternalInput")
    t_y = nc.dram_tensor("y", [NBLK, 1, BLK], F32, kind="ExternalOutput")

    with tile.TileContext(nc) as tc:
        with (
            tc.tile_pool(name="wpool", bufs=1) as wpool,
            tc.tile_pool(name="cpool", bufs=12) as cpool,
            tc.tile_pool(name="xpool", bufs=2) as xpool,
            tc.tile_pool(name="hpool", bufs=2) as hpool,
            tc.tile_pool(name="fpool", bufs=2) as fpool,
            tc.tile_pool(name="opool", bufs=2) as opool,
            tc.tile_pool(name="pt_ps", bufs=2, space="PSUM") as pt_ps,
            tc.tile_pool(name="mm_ps", bufs=2, space="PSUM") as mm_ps,
            tc.tile_pool(name="o_ps", bufs=1, space="PSUM") as o_ps,
        ):
            # ---- loads (idx first: it gates the gather stream) ----
            idx_all = wpool.tile([128, F * (BC // 16)], I16)
            nc.sync.dma_start(out=idx_all[:], in_=t_idx[:])
            mask_sb = wpool.tile([128, 4 * NSB * F], U8)
            nc.sync.dma_start(out=mask_sb[:], in_=t_mask[:])
            mask4 = mask_sb[:].rearrange("p (r k f) -> p r k f", r=4, f=F)

            # whole-core X tile [128, 16 sub-blocks, 455]
            xball = wpool.tile([128, NSB * XW], BF16)
            xb3 = xball[:].rearrange("p (k w) -> p k w", w=XW)
            cont_src = t_cont[:].rearrange("(k p) c -> p k c", p=SUB)
            nc.sync.dma_start(out=xb3[:, :, 0:NCONT], in_=cont_src)

            ident = wpool.tile([128, 128], BF16)
            make_identity(nc, ident)
            identf = wpool.tile([128, 128], F32)
            make_identity(nc, identf)

            w1_sb = []
            for ci, (k0, ks) in enumerate(KCH):
                w1c = wpool.tile([128, H1], BF16, name=f"w1c{ci}")
                nc.sync.dma_start(out=w1c[0:ks, :], in_=t_w1[k0 : k0 + ks, :])
                w1_sb.append(w1c)
            w2_sb = []
            for ci, (k0, ks) in enumerate(MCH1):
                w2c = wpool.tile([128, H2], BF16, name=f"w2c{ci}")
                nc.sync.dma_start(out=w2c[0:ks, :], in_=t_w2[k0 : k0 + ks, :])
                w2_sb.append(w2c)
            b1_sb = []
            for mi, (m0, ms) in enumerate(MCH1):
                b1m = wpool.tile([128, 1], F32, name=f"b1m{mi}")
                nc.sync.dma_start(out=b1m[0:ms, :], in_=t_b1[m0 : m0 + ms, :])
                b1_sb.append(b1m)
            b2_sb = []
            for mi, (m0, ms) in enumerate(MCH2):
                b2m = wpool.tile([128, 1], F32, name=f"b2m{mi}")
                nc.sync.dma_start(out=b2m[0:ms, :], in_=t_b2[m0 : m0 + ms, :])
                b2_sb.append(b2m)
            wo_sb = wpool.tile([128, n_wo_ch], BF16)
            nc.sync.dma_start(out=wo_sb[:], in_=t_wo[:])
            wc_sb = wpool.tile([128, NCONT], F32)
            nc.sync.dma_start(out=wc_sb[:], in_=t_wc[:])
            fs_sb = wpool.tile([128, 1], F32)
            nc.sync.dma_start(out=fs_sb[:], in_=t_fs[:])
            ob_sb = wpool.tile([1, 1], F32)
            nc.sync.dma_start(out=ob_sb[:], in_=t_ob[:])

            # ---- gather + select: 2 chunks of 1024 per feature, 4 queues ----
            NGC = BC // NG            # idxs per gather chunk (1024)
            KC = NGC // SUB           # sub-blocks per chunk (8)
            for ch in range(NG):
                forder = list(range(F)) if ch == 0 else list(range(F - 1, -1, -1))
                for fi, f in enumerate(forder):
                    g = ch * F + fi
                    cf = cpool.tile([128, KC * EQ], BF16, tag="cf")
                    col0 = (g) * (NGC // 16)
                    nc.gpsimd.dma_gather(
                        out_ap=cf[:].rearrange("p (k e) -> p k e", e=EQ),
                        in_ap=t_table[f * VQ : (f + 1) * VQ, :],
                        idxs_ap=idx_all[:, col0 : col0 + NGC // 16],
                        num_idxs=NGC,
                        num_idxs_reg=NGC,
                        elem_size=EQ,
                        queue_num=QASSIGN[g],
                    )
                    c3 = cf[:].rearrange("p (k e) -> p k e", e=EQ)
                    xout = xb3[:, ch * KC : (ch + 1) * KC,
                               NCONT + W17 * f : NCONT + W17 * (f + 1)]
                    nc.vector.tensor_copy(out=xout, in_=c3[:, :, 0:W17])
                    for r in range(1, 4):
                        nc.vector.copy_predicated(
                            out=xout,
                            mask=mask4[:, r, ch * KC : (ch + 1) * KC,
                                       f : f + 1].to_broadcast([128, KC, W17]),
                            data=c3[:, :, 32 * r : 32 * r + W17],
                        )

            # ---- per-block MLP + FM ----
            for blk in range(NBLK):
                xslab = xball[:, blk * NSUB * XW : (blk + 1) * NSUB * XW]
                xs3 = xb3[:, blk * NSUB : (blk + 1) * NSUB, :]

                corder = (list(range(len(KCH))) if blk < NBLK // 2
                          else list(range(len(KCH) - 1, -1, -1)))
                xt_sb = [None] * len(KCH)
                for ci in corder:
                    k0, ks = KCH[ci]
                    pt = pt_ps.tile([128, BLK], BF16, tag="pt")
                    for s in range(NSUB):
                        nc.tensor.transpose(
                            out=pt[0:ks, s * SUB : (s + 1) * SUB],
                            in_=xslab[:, s * XW + k0 : s * XW + k0 + ks],
                            identity=ident[:],
                        )
                    xt = xpool.tile([128, BLK], BF16, tag=f"xt{ci}")
                    nc.scalar.copy(out=xt[0:ks, :], in_=pt[0:ks, :])
                    xt_sb[ci] = xt

                fmv_sb = []
                for s in range(NSUB):
                    g3 = xs3[:, s, NCONT:XW].rearrange("p (f w) -> p f w", w=W17)
                    emb_fd = g3[:, :, 0:D]
                    emb_df = xs3[:, s, NCONT:XW].rearrange(
                        "p (f w) -> p w f", w=W17
                    )[:, 0:D, :]
                    first_f = g3[:, :, D : D + 1].rearrange("p f w -> p (f w)")

                    se = fpool.tile([SUB, D], F32, tag="se")
                    nc.vector.tensor_reduce(
                        out=se[:], in_=emb_df, axis=mybir.AxisListType.X, op=ALU.add
                    )
                    se2 = fpool.tile([SUB, D], F32, tag="se2")
                    nc.vector.tensor_mul(out=se2[:], in0=se[:], in1=se[:])
                    r1 = fpool.tile([SUB, 1], F32, tag="r1")
                    nc.vector.tensor_reduce(
                        out=r1[:], in_=se2[:], axis=mybir.AxisListType.X, op=ALU.add
                    )
                    sq = fpool.tile([SUB, F * D], F32, tag="sq")
                    nc.vector.tensor_mul(
                        out=sq[:].rearrange("p (f w) -> p f w", w=D),
                        in0=emb_fd, in1=emb_fd)
                    r2 = fpool.tile([SUB, 1], F32, tag="r2")
                    nc.vector.tensor_reduce(
                        out=r2[:], in_=sq[:], axis=mybir.AxisListType.X, op=ALU.add
                    )
                    rf = fpool.tile([SUB, 1], F32, tag="rf")
                    nc.vector.tensor_reduce(
                        out=rf[:], in_=first_f, axis=mybir.AxisListType.X, op=ALU.add
                    )
                    cw = fpool.tile([SUB, NCONT], F32, tag="cw")
                    nc.vector.tensor_mul(
                        out=cw[:], in0=xs3[:, s, 0:NCONT], in1=wc_sb[:])
                    r3 = fpool.tile([SUB, 1], F32, tag="r3")
                    nc.vector.tensor_reduce(
                        out=r3[:], in_=cw[:], axis=mybir.AxisListType.X, op=ALU.add
                    )
                    t1 = fpool.tile([SUB, 1], F32, tag="t1")
                    nc.vector.tensor_sub(out=t1[:], in0=r1[:], in1=r2[:])
                    t2 = fpool.tile([SUB, 1], F32, tag="t2")
                    nc.vector.tensor_scalar_mul(out=t2[:], in0=t1[:], scalar1=0.5)
                    t3 = fpool.tile([SUB, 1], F32, tag="t3")
                    nc.vector.tensor_add(out=t3[:], in0=t2[:], in1=r3[:])
                    t4 = fpool.tile([SUB, 1], F32, tag="t4")
                    nc.vector.tensor_add(out=t4[:], in0=t3[:], in1=rf[:])
                    fmv = fpool.tile([SUB, 1], F32, tag=f"fmv{s}")
                    nc.vector.tensor_mul(out=fmv[:], in0=t4[:], in1=fs_sb[:])
                    fmv_sb.append(fmv)

                h1_sb = []
                for mi, (m0, ms) in enumerate(MCH1):
                    ps1 = mm_ps.tile([128, BLK], F32, tag="ps1")
                    for oi, ci in enumerate(corder):
                        k0, ks = KCH[ci]
                        nc.tensor.matmul(
                            out=ps1[0:ms, :],
                            lhsT=w1_sb[ci][0:ks, m0 : m0 + ms],
                            rhs=xt_sb[ci][0:ks, :],
                            start=(oi == 0), stop=(oi == len(KCH) - 1),
                        )
                    h1m = hpool.tile([128, BLK], BF16, tag=f"h1m{mi}")
                    nc.scalar.activation(
                        out=h1m[0:ms, :], in_=ps1[0:ms, :], func=AF.Relu,
                        bias=b1_sb[mi][0:ms, :],
                    )
                    h1_sb.append(h1m)

                h2_sb = []
                for mi, (m0, ms) in enumerate(MCH2):
                    ps2 = mm_ps.tile([128, BLK], F32, tag="ps2")
                    for ci, (k0, ks) in enumerate(MCH1):
                        nc.tensor.matmul(
                            out=ps2[0:ms, :],
                            lhsT=w2_sb[ci][0:ks, m0 : m0 + ms],
                            rhs=h1_sb[ci][0:ks, :],
                            start=(ci == 0), stop=(ci == len(MCH1) - 1),
                        )
                    h2m = hpool.tile([128, BLK], BF16, tag=f"h2m{mi}")
                    nc.scalar.activation(
                        out=h2m[0:ms, :], in_=ps2[0:ms, :], func=AF.Relu,
                        bias=b2_sb[mi][0:ms, :],
                    )
                    h2_sb.append(h2m)

                pso = o_ps.tile([1, BLK], F32, tag="pso")
                for ci, (k0, ks) in enumerate(MCH2):
                    nc.tensor.matmul(
                        out=pso[0:1, :],
                        lhsT=wo_sb[0:ks, ci : ci + 1],
                        rhs=h2_sb[ci][0:ks, :],
                        start=(ci == 0), stop=(ci == len(MCH2) - 1),
                    )
                pft = o_ps.tile([1, BLK], F32, tag="pft")
                for s in range(NSUB):
                    nc.tensor.transpose(
                        out=pft[0:1, s * SUB : (s + 1) * SUB],
                        in_=fmv_sb[s][:, 0:1],
                        identity=identf[:],
                    )
                fsb = opool.tile([1, BLK], F32, tag="fsb")
                nc.scalar.copy(out=fsb[:], in_=pft[0:1, :])
                orow = opool.tile([1, BLK], F32, tag="orow")
                nc.scalar.activation(
                    out=orow[:], in_=pso[0:1, :], func=AF.Identity,
                    bias=ob_sb[0:1, :],
                )
                oout = opool.tile([1, BLK], F32, tag="oout")
                nc.vector.tensor_add(out=oout[:], in0=orow[:], in1=fsb[:])
                nc.sync.dma_start(out=t_y[blk], in_=oout[:])

    nc.compile()
    return nc


def prep_inputs(continuous, cat_idx, W_cont, b_cont, emb_first, emb, W1, b1,
                W2, b2, W_out, b_out):
    # quad table [F*VQ, 128]: 4 entries x [16 emb | first | 15 pad]
    tabq = np.zeros((F, VQ, 4, 32), np.float32)
    tabq[:, :, :, 0:D] = np.asarray(emb, np.float32).reshape(F, VQ, 4, D)
    tabq[:, :, :, D] = np.asarray(emb_first, np.float32).reshape(F, VQ, 4)
    tabq = tabq.reshape(F * VQ, EQ).astype(ml_dtypes.bfloat16)

    cat = np.asarray(cat_idx).astype(np.int64)      # [B, F]
    quad = (cat >> 2).astype(np.int16)
    lo = (cat & 3).astype(np.int64)

    W1 = np.asarray(W1, np.float32)
    w1p = np.zeros((XW, H1), np.float32)
    w1p[0:NCONT] = W1[0:NCONT]
    for ff in range(F):
        w1p[NCONT + W17 * ff : NCONT + W17 * ff + D] = (
            W1[NCONT + D * ff : NCONT + D * ff + D])

    W_out = np.asarray(W_out, np.float32)
    n_wo_ch = (H2 + 127) // 128
    wo_t = np.zeros((n_wo_ch, 128), np.float32)
    wo_t.reshape(-1)[:H2] = W_out[1:, 0]
    wo = np.ascontiguousarray(wo_t.T)

    w_fm = np.float32(W_out[0, 0])
    ob = np.float32(b_out[0] + w_fm * b_cont[0])

    common = {
        "table": tabq,
        "w1p": w1p.astype(ml_dtypes.bfloat16),
        "w2": np.asarray(W2, np.float32).astype(ml_dtypes.bfloat16),
        "b1": np.asarray(b1, np.float32).reshape(H1, 1),
        "b2": np.asarray(b2, np.float32).reshape(H2, 1),
        "wo": wo.astype(ml_dtypes.bfloat16),
        "wc": np.tile(np.asarray(W_cont, np.float32).reshape(1, NCONT), (128, 1)),
        "fs": np.full((128, 1), w_fm, np.float32),
        "ob": np.array([[ob]], np.float32),
    }

    in_maps = []
    for c in range(NCORES):
        rows = slice(c * BC, (c + 1) * BC)
        # [NSB, 128, F] per-core views, sub-block-major
        qc = quad[rows].reshape(NSB, SUB, F)
        loc = lo[rows].reshape(NSB, SUB, F)

        # idx buffer: gather g=(ch,f), seq i = k_local*128 + p ->
        # [QPART[q] + i%16, g*64 + i//16]
        NGC = BC // NG
        idx_buf = np.zeros((128, F * (BC // 16)), np.int16)
        for ch in range(NG):
            forder = list(range(F)) if ch == 0 else list(range(F - 1, -1, -1))
            for fi, ff in enumerate(forder):
                g = ch * F + fi
                q = QASSIGN[g]
                flat = qc[ch * (NSB // NG) : (ch + 1) * (NSB // NG), :, ff].reshape(-1)
                p0 = QPART[q]
                idx_buf[p0 : p0 + 16, g * (NGC // 16) : (g + 1) * (NGC // 16)] = (
                    flat.reshape(NGC // 16, 16).T)

        # masks [128, 4, NSB, F] bf16
        m = np.zeros((4, NSB, SUB, F), np.float32)
        for r in range(4):
            m[r] = (loc == r)
        m = m.transpose(2, 0, 1, 3).reshape(128, 4 * NSB * F)

        in_maps.append({
            **common,
            "idx": idx_buf,
            "mask": m.astype(np.uint8),
            "cont": np.asarray(continuous[rows], np.float32).astype(
                ml_dtypes.bfloat16),
        })
    return in_maps


_NC_CACHE = {}


def kernel(**inputs) -> np.ndarray:
    if "nc" not in _NC_CACHE:
        _NC_CACHE["nc"] = build_kernel()
    nc = _NC_CACHE["nc"]
    in_maps = prep_inputs(**inputs)
    res = run_bass_kernel_spmd(nc, in_maps, core_ids=list(range(NCORES)))
    out = np.concatenate(
        [r["y"].reshape(BC, 1) for r in res.results], axis=0)
    return out.astype(np.float32)



# revision 8
# speedup vs baseline: 1.2587x; 1.2587x over previous
"""DeepFM (embedding_lookup) Trainium2 Bass kernel.

Gather strategy: the embedding+first table is stored as 256-byte "quad"
rows (4 consecutive vocab entries x 32 bf16 each: [16 emb | first | 15
pad]). Per feature, one InstDMAGatherAnt gathers all samples' quads
(int16 quad index = v>>2 fits: 25000 < 32767). The right entry of the 4
candidates is selected on-chip with host-built masks via
copy_predicated, batched 4 features per op group.

FM first+second order terms are computed entirely on the PE as constant
matmuls accumulated into the output PSUM row:
  y = ob + wlin^T X + wq^T (X*X) + shalf^T (A^T X)^2 + Wout^T h
where X is the transposed [455, rows] input slab, wlin holds
w_fm*(W_cont | per-f first-column ones), wq = -0.5*w_fm at emb
positions, A sums emb dims over features, shalf = 0.5*w_fm.

Data-parallel on batch across 8 cores; table replicated.
"""

import numpy as np
import ml_dtypes

import concourse.bass as bass
import concourse.bacc as bacc
import concourse.mybir as mybir
import concourse.tile as tile
from concourse.bass_utils import run_bass_kernel_spmd
from concourse.masks import make_identity

F32 = mybir.dt.float32
BF16 = mybir.dt.bfloat16
I32 = mybir.dt.int32
I16 = mybir.dt.int16
U8 = mybir.dt.uint8
AF = mybir.ActivationFunctionType
ALU = mybir.AluOpType

B, NCONT, F, V, D = 16384, 13, 26, 100000, 16
H1, H2 = 400, 400
NCORES = 8
BC = B // NCORES          # 2048 rows per core
SUB = 128
NSUB = 4
BLK = SUB * NSUB          # 512
NBLK = BC // BLK          # 4
NSB = NBLK * NSUB         # 16 sub-blocks of 128 rows per core
W17 = D + 1
GW = F * W17              # 442
XW = NCONT + GW           # 455
VQ = V // 4               # 25000 quad rows per feature
EQ = 128                  # quad row: 4 x 32 bf16 = 256B
NG = 2                    # gather chunks per feature (1024 idxs each)
NGATHER = NG * F          # 52 gathers per core
FG = 4                    # features per select group
QASSIGN = [g % 4 for g in range(NGATHER)]
# idx partition base per queue (probed on HW): queue q reads partitions
# QPART[q] .. QPART[q]+15 of the idxs AP
QPART = [16, 48, 80, 112]
# feature groups: [(f0, nf), ...] covering 26 features
FGROUPS = [(f0, min(FG, F - f0)) for f0 in range(0, F, FG)]


def _chunks(total, step=128):
    return [(s, min(step, total - s)) for s in range(0, total, step)]


def build_kernel():
    KCH = _chunks(XW)
    MCH1 = _chunks(H1)
    MCH2 = _chunks(H2)
    n_wo_ch = len(MCH2)

    nc = bacc.Bacc("TRN2", target_bir_lowering=False, debug=False,
                   dynamic_dma_scratch_size=32768, num_swdge_queues=4)

    t_table = nc.dram_tensor("table", [F * VQ, EQ], BF16, kind="ExternalInput")
    t_idx = nc.dram_tensor("idx", [128, F * (BC // 16)], I16, kind="ExternalInput")
    t_mask = nc.dram_tensor("mask", [128, 4 * NSB * F], U8, kind="ExternalInput")
    t_cont = nc.dram_tensor("cont", [BC, NCONT], BF16, kind="ExternalInput")
    t_w1 = nc.dram_tensor("w1p", [XW, H1], BF16, kind="ExternalInput")
    t_w2 = nc.dram_tensor("w2", [H1, H2], BF16, kind="ExternalInput")
    t_b1 = nc.dram_tensor("b1", [H1, 1], F32, kind="ExternalInput")
    t_b2 = nc.dram_tensor("b2", [H2, 1], F32, kind="ExternalInput")
    t_wo = nc.dram_tensor("wo", [128, n_wo_ch], BF16, kind="ExternalInput")
    t_wlin = nc.dram_tensor("wlin", [128, len(KCH)], BF16, kind="ExternalInput")
    t_wq = nc.dram_tensor("wq", [128, len(KCH)], BF16, kind="ExternalInput")
    t_amat = nc.dram_tensor("amat", [128, len(KCH) * D], BF16, kind="ExternalInput")
    t_shalf = nc.dram_tensor("shalf", [128, 1], BF16, kind="ExternalInput")
    t_ob = nc.dram_tensor("ob", [1, 1], F32, kind="ExternalInput")
    t_y = nc.dram_tensor("y", [NBLK, 1, BLK], F32, kind="ExternalOutput")

    with tile.TileContext(nc) as tc:
        with (
            tc.tile_pool(name="wpool", bufs=1) as wpool,
            tc.tile_pool(name="cpool", bufs=7) as cpool,
            tc.tile_pool(name="xpool", bufs=8) as xpool,
            tc.tile_pool(name="qpool", bufs=4) as qpool,
            tc.tile_pool(name="hpool", bufs=4) as hpool,
            tc.tile_pool(name="spool", bufs=2) as spool,
            tc.tile_pool(name="opool", bufs=2) as opool,
            tc.tile_pool(name="pt_ps", bufs=2, space="PSUM") as pt_ps,
            tc.tile_pool(name="mm_ps", bufs=2, space="PSUM") as mm_ps,
            tc.tile_pool(name="s_ps", bufs=1, space="PSUM") as s_ps,
            tc.tile_pool(name="o_ps", bufs=1, space="PSUM") as o_ps,
        ):
            # ---- loads (idx first: it gates the gather stream) ----
            idx_all = wpool.tile([128, F * (BC // 16)], I16)
            # first 4 gathers' columns land fast; the rest follow
            nc.sync.dma_start(out=idx_all[:, 0:256], in_=t_idx[:, 0:256])
            nc.sync.dma_start(out=idx_all[:, 256:], in_=t_idx[:, 256:])
            mask_sb = wpool.tile([128, 4 * NSB * F], U8)
            nc.sync.dma_start(out=mask_sb[:], in_=t_mask[:])
            mask4 = mask_sb[:].rearrange("p (r k f) -> p r k f", r=4, f=F)

            # whole-core X tile [128, 16 sub-blocks, 455]
            xball = wpool.tile([128, NSB * XW], BF16)
            xb3 = xball[:].rearrange("p (k w) -> p k w", w=XW)
            cont_src = t_cont[:].rearrange("(k p) c -> p k c", p=SUB)
            nc.sync.dma_start(out=xb3[:, :, 0:NCONT], in_=cont_src)

            ident = wpool.tile([128, 128], BF16)
            make_identity(nc, ident)

            w1_sb = []
            for ci, (k0, ks) in enumerate(KCH):
                w1c = wpool.tile([128, H1], BF16, name=f"w1c{ci}")
                nc.sync.dma_start(out=w1c[0:ks, :], in_=t_w1[k0 : k0 + ks, :])
                w1_sb.append(w1c)
            w2_sb = []
            for ci, (k0, ks) in enumerate(MCH1):
                w2c = wpool.tile([128, H2], BF16, name=f"w2c{ci}")
                nc.sync.dma_start(out=w2c[0:ks, :], in_=t_w2[k0 : k0 + ks, :])
                w2_sb.append(w2c)
            b1_sb = []
            for mi, (m0, ms) in enumerate(MCH1):
                b1m = wpool.tile([128, 1], F32, name=f"b1m{mi}")
                nc.sync.dma_start(out=b1m[0:ms, :], in_=t_b1[m0 : m0 + ms, :])
                b1_sb.append(b1m)
            b2_sb = []
            for mi, (m0, ms) in enumerate(MCH2):
                b2m = wpool.tile([128, 1], F32, name=f"b2m{mi}")
                nc.sync.dma_start(out=b2m[0:ms, :], in_=t_b2[m0 : m0 + ms, :])
                b2_sb.append(b2m)
            wo_sb = wpool.tile([128, n_wo_ch], BF16)
            nc.sync.dma_start(out=wo_sb[:], in_=t_wo[:])
            wlin_sb = wpool.tile([128, len(KCH)], BF16)
            nc.sync.dma_start(out=wlin_sb[:], in_=t_wlin[:])
            wq_sb = wpool.tile([128, len(KCH)], BF16)
            nc.sync.dma_start(out=wq_sb[:], in_=t_wq[:])
            amat_sb = wpool.tile([128, len(KCH) * D], BF16)
            nc.sync.dma_start(out=amat_sb[:], in_=t_amat[:])
            shalf_sb = wpool.tile([128, 1], BF16)
            nc.sync.dma_start(out=shalf_sb[:], in_=t_shalf[:])
            ob_sb = wpool.tile([1, 1], F32)
            nc.sync.dma_start(out=ob_sb[:], in_=t_ob[:])

            NGC = BC // NG            # idxs per gather chunk (1024)
            KC = NGC // SUB           # sub-blocks per chunk (8)

            def emit_gathers(ch):
                """Gather + select for one half (sub-blocks ch*8..ch*8+7)."""
                for f0, nf in FGROUPS:
                    cf = cpool.tile([128, FG * KC * EQ], BF16, tag="cf")
                    c4 = cf[:].rearrange("p (f k e) -> p f k e", f=FG, e=EQ)
                    for fi in range(nf):
                        f = f0 + fi
                        g = ch * F + f
                        col0 = g * (NGC // 16)
                        nc.gpsimd.dma_gather(
                            out_ap=c4[:, fi],
                            in_ap=t_table[f * VQ : (f + 1) * VQ, :],
                            idxs_ap=idx_all[:, col0 : col0 + NGC // 16],
                            num_idxs=NGC,
                            num_idxs_reg=NGC,
                            elem_size=EQ,
                            queue_num=QASSIGN[g],
                        )
                    # batched select: in [p, f, k, 0:17] -> out [p, k, f, 17]
                    src = c4[:, 0:nf, :, 0:W17]
                    xout = xb3[:, ch * KC : (ch + 1) * KC,
                               NCONT + W17 * f0 : NCONT + W17 * (f0 + nf)
                               ].rearrange("p k (f w) -> p f k w", w=W17)
                    nc.scalar.copy(out=xout, in_=src)
                    for r in range(1, 4):
                        m = mask4[:, r, ch * KC : (ch + 1) * KC, f0 : f0 + nf]
                        nc.vector.copy_predicated(
                            out=xout.rearrange("p f k w -> p k f w"),
                            mask=m.to_broadcast([128, KC, nf, W17]),
                            data=c4[:, 0:nf, :, 32 * r : 32 * r + W17
                                    ].rearrange("p f k w -> p k f w"),
                        )

            def emit_block(blk):
                xslab = xball[:, blk * NSUB * XW : (blk + 1) * NSUB * XW]

                xt_sb = []
                for ci, (k0, ks) in enumerate(KCH):
                    pt = pt_ps.tile([128, BLK], BF16, tag="pt")
                    for s in range(NSUB):
                        nc.tensor.transpose(
                            out=pt[0:ks, s * SUB : (s + 1) * SUB],
                            in_=xslab[:, s * XW + k0 : s * XW + k0 + ks],
                            identity=ident[:],
                        )
                    xt = xpool.tile([128, BLK], BF16, tag=f"xt{ci}")
                    nc.scalar.copy(out=xt[0:ks, :], in_=pt[0:ks, :])
                    xt_sb.append(xt)

                # fm: accumulate everything into one [1, BLK] psum row
                po = o_ps.tile([1, BLK], F32, tag="po")
                # wlin over chunks
                for ci, (k0, ks) in enumerate(KCH):
                    nc.tensor.matmul(
                        out=po[0:1, :], lhsT=wlin_sb[0:ks, ci : ci + 1],
                        rhs=xt_sb[ci][0:ks, :],
                        start=(ci == 0), stop=False,
                    )
                # squared input, wq over chunks
                xsq_sb = []
                for ci, (k0, ks) in enumerate(KCH):
                    xsq = qpool.tile([128, BLK], BF16, tag=f"xsq{ci}")
                    nc.vector.tensor_mul(
                        out=xsq[0:ks, :], in0=xt_sb[ci][0:ks, :],
                        in1=xt_sb[ci][0:ks, :])
                    xsq_sb.append(xsq)
                    nc.tensor.matmul(
                        out=po[0:1, :], lhsT=wq_sb[0:ks, ci : ci + 1],
                        rhs=xsq[0:ks, :], start=False, stop=False,
                    )
                # S = A^T X  [16, BLK]
                ps = s_ps.tile([D, BLK], F32, tag="ps")
                for ci, (k0, ks) in enumerate(KCH):
                    nc.tensor.matmul(
                        out=ps[0:D, :],
                        lhsT=amat_sb[0:ks, ci * D : (ci + 1) * D],
                        rhs=xt_sb[ci][0:ks, :],
                        start=(ci == 0), stop=(ci == len(KCH) - 1),
                    )
                s2 = spool.tile([D, BLK], BF16, tag="s2")
                nc.scalar.activation(out=s2[:], in_=ps[0:D, :], func=AF.Square)
                nc.tensor.matmul(
                    out=po[0:1, :], lhsT=shalf_sb[0:D, 0:1], rhs=s2[:],
                    start=False, stop=False,
                )

                h1_sb = []
                for mi, (m0, ms) in enumerate(MCH1):
                    ps1 = mm_ps.tile([128, BLK], F32, tag="ps1")
                    for ci, (k0, ks) in enumerate(KCH):
                        nc.tensor.matmul(
                            out=ps1[0:ms, :],
                            lhsT=w1_sb[ci][0:ks, m0 : m0 + ms],
                            rhs=xt_sb[ci][0:ks, :],
                            start=(ci == 0), stop=(ci == len(KCH) - 1),
                        )
                    h1m = hpool.tile([128, BLK], BF16, tag=f"h1m{mi}")
                    nc.scalar.activation(
                        out=h1m[0:ms, :], in_=ps1[0:ms, :], func=AF.Relu,
                        bias=b1_sb[mi][0:ms, :],
                    )
                    h1_sb.append(h1m)

                h2_sb = []
                for mi, (m0, ms) in enumerate(MCH2):
                    ps2 = mm_ps.tile([128, BLK], F32, tag="ps2")
                    for ci, (k0, ks) in enumerate(MCH1):
                        nc.tensor.matmul(
                            out=ps2[0:ms, :],
                            lhsT=w2_sb[ci][0:ks, m0 : m0 + ms],
                            rhs=h1_sb[ci][0:ks, :],
                            start=(ci == 0), stop=(ci == len(MCH1) - 1),
                        )
                    h2m = hpool.tile([128, BLK], BF16, tag=f"h2m{mi}")
                    nc.scalar.activation(
                        out=h2m[0:ms, :], in_=ps2[0:ms, :], func=AF.Relu,
                        bias=b2_sb[mi][0:ms, :],
                    )
                    h2_sb.append(h2m)

                for ci, (k0, ks) in enumerate(MCH2):
                    nc.tensor.matmul(
                        out=po[0:1, :],
                        lhsT=wo_sb[0:ks, ci : ci + 1],
                        rhs=h2_sb[ci][0:ks, :],
                        start=False, stop=(ci == len(MCH2) - 1),
                    )
                orow = opool.tile([1, BLK], F32, tag="orow")
                nc.scalar.activation(
                    out=orow[:], in_=po[0:1, :], func=AF.Identity,
                    bias=ob_sb[0:1, :],
                )
                nc.sync.dma_start(out=t_y[blk], in_=orow[:])

            # pipeline: gathers(ch0) | blocks 0-1 overlap gathers(ch1) | blocks 2-3
            emit_gathers(0)
            emit_block(0)
            emit_block(1)
            emit_gathers(1)
            emit_block(2)
            emit_block(3)

    nc.compile()
    return nc


def prep_inputs(continuous, cat_idx, W_cont, b_cont, emb_first, emb, W1, b1,
                W2, b2, W_out, b_out):
    KCH = _chunks(XW)
    # quad table [F*VQ, 128]: 4 entries x [16 emb | first | 15 pad]
    tabq = np.zeros((F, VQ, 4, 32), np.float32)
    tabq[:, :, :, 0:D] = np.asarray(emb, np.float32).reshape(F, VQ, 4, D)
    tabq[:, :, :, D] = np.asarray(emb_first, np.float32).reshape(F, VQ, 4)
    tabq = tabq.reshape(F * VQ, EQ).astype(ml_dtypes.bfloat16)

    cat = np.asarray(cat_idx).astype(np.int64)      # [B, F]
    quad = (cat >> 2).astype(np.int16)
    lo = (cat & 3).astype(np.int64)

    W1 = np.asarray(W1, np.float32)
    w1p = np.zeros((XW, H1), np.float32)
    w1p[0:NCONT] = W1[0:NCONT]
    for ff in range(F):
        w1p[NCONT + W17 * ff : NCONT + W17 * ff + D] = (
            W1[NCONT + D * ff : NCONT + D * ff + D])

    W_out = np.asarray(W_out, np.float32)
    n_wo_ch = (H2 + 127) // 128
    wo_t = np.zeros((n_wo_ch, 128), np.float32)
    wo_t.reshape(-1)[:H2] = W_out[1:, 0]
    wo = np.ascontiguousarray(wo_t.T)

    w_fm = np.float32(W_out[0, 0])
    ob = np.float32(b_out[0] + w_fm * b_cont[0])

    # FM constant matrices (chunked by KCH along the 455-dim axis)
    wlin_full = np.zeros((XW,), np.float32)
    wlin_full[0:NCONT] = np.asarray(W_cont, np.float32).reshape(-1) * w_fm
    wq_full = np.zeros((XW,), np.float32)
    amat_full = np.zeros((XW, D), np.float32)
    for ff in range(F):
        base = NCONT + W17 * ff
        wlin_full[base + D] = w_fm                     # first-order column
        wq_full[base : base + D] = -0.5 * w_fm          # -0.5 sum e^2
        amat_full[base : base + D, :] = np.eye(D, dtype=np.float32)
    wlin_t = np.zeros((128, len(KCH)), np.float32)
    wq_t = np.zeros((128, len(KCH)), np.float32)
    amat_t = np.zeros((128, len(KCH) * D), np.float32)
    for ci, (k0, ks) in enumerate(KCH):
        wlin_t[0:ks, ci] = wlin_full[k0 : k0 + ks]
        wq_t[0:ks, ci] = wq_full[k0 : k0 + ks]
        amat_t[0:ks, ci * D : (ci + 1) * D] = amat_full[k0 : k0 + ks, :]
    shalf_t = np.zeros((128, 1), np.float32)
    shalf_t[0:D, 0] = 0.5 * w_fm

    common = {
        "table": tabq,
        "w1p": w1p.astype(ml_dtypes.bfloat16),
        "w2": np.asarray(W2, np.float32).astype(ml_dtypes.bfloat16),
        "b1": np.asarray(b1, np.float32).reshape(H1, 1),
        "b2": np.asarray(b2, np.float32).reshape(H2, 1),
        "wo": wo.astype(ml_dtypes.bfloat16),
        "wlin": wlin_t.astype(ml_dtypes.bfloat16),
        "wq": wq_t.astype(ml_dtypes.bfloat16),
        "amat": amat_t.astype(ml_dtypes.bfloat16),
        "shalf": shalf_t.astype(ml_dtypes.bfloat16),
        "ob": np.array([[ob]], np.float32),
    }

    in_maps = []
    for c in range(NCORES):
        rows = slice(c * BC, (c + 1) * BC)
        # [NSB, 128, F] per-core views, sub-block-major
        qc = quad[rows].reshape(NSB, SUB, F)
        loc = lo[rows].reshape(NSB, SUB, F)

        # idx buffer: gather g=(ch,f), seq i = k_local*128 + p ->
        # [QPART[q] + i%16, g*64 + i//16]
        NGC = BC // NG
        idx_buf = np.zeros((128, F * (BC // 16)), np.int16)
        for ch in range(NG):
            for ff in range(F):
                g = ch * F + ff
                q = QASSIGN[g]
                flat = qc[ch * (NSB // NG) : (ch + 1) * (NSB // NG), :, ff].reshape(-1)
                p0 = QPART[q]
                idx_buf[p0 : p0 + 16, g * (NGC // 16) : (g + 1) * (NGC // 16)] = (
                    flat.reshape(NGC // 16, 16).T)

        # masks [128, 4, NSB, F] u8
        m = np.zeros((4, NSB, SUB, F), np.float32)
        for r in range(4):
            m[r] = (loc == r)
        m = m.transpose(2, 0, 1, 3).reshape(128, 4 * NSB * F)

        in_maps.append({
            **common,
            "idx": idx_buf,
            "mask": m.astype(np.uint8),
            "cont": np.asarray(continuous[rows], np.float32).astype(
                ml_dtypes.bfloat16),
        })
    return in_maps


_NC_CACHE = {}


def kernel(**inputs) -> np.ndarray:
    if "nc" not in _NC_CACHE:
        _NC_CACHE["nc"] = build_kernel()
    nc = _NC_CACHE["nc"]
    in_maps = prep_inputs(**inputs)
    res = run_bass_kernel_spmd(nc, in_maps, core_ids=list(range(NCORES)))
    out = np.concatenate(
        [r["y"].reshape(BC, 1) for r in res.results], axis=0)
    return out.astype(np.float32)
